# revision 8
# baseline (speedup 1.0000x reference)
"""GQA attention kernel for 8 TRN2 NeuronCores.

Problem: B=2, T=2048, D=2048, H=16 q-heads, KV=4 kv-heads, HD=128, RoPE,
non-causal softmax, out projection. f32 reference, rel-err gate 2e-2.

Sharding: 8 cores = 2 batches x 4 kv-groups. Core c handles batch c//4 and
kv-group c%4 (4 q heads + 1 kv head). Each core computes a partial output
x @ wq_g -> attention -> (heads g) @ wo_g^T: full [T, D] partial summed on
host over the 4 groups of each batch (tensor-parallel unshard).

On-device layout: everything transposed ([hd, t], hd=128=partition dim).
All matmul operands are bf16 (HW forbids mixing 32-bit with 16-bit
operands): halved input DMA, Fast Weight Load on every stationary
operand, and 2x DVE throughput for the bf16 softmax-denominator
accumulation. The output is DMA'd as bf16 and accumulated in f32 on the
host (halves output traffic; ~1e-4 extra error). fp8/DoubleRow was
measured numerically unusable (>2.5% error per quantized operand pair
vs the 2e-2 budget); this all-bf16 layout lands at ~9.7e-3.

Engine placement (measured, not theoretical): PE runs all matmuls
(~270us busy of ~301us span, the binding resource); ACT runs only the
softmax exp (135us; its ~1054ns/tile paces the attention inner loop, so
out-projection pieces are interleaved 4x per head as PE filler); DVE
does RoPE, denominator adds, normalize, and all PSUM evacuation. The
Pool engine is left idle: on HW it takes ~2.2us per 1024-elem tensor op
(sim models it fast) and it cannot access PSUM. All output DMA triggers
ride the SP queue - triggers on the scalar queue serialize with ACT's
exp instruction stream.
"""
import os
import sys

for _p in ("/opt/trn_rl_repo", "/root/.axon_site/_ro/trn_rl_repo"):
    if os.path.isdir(_p) and _p not in sys.path:
        sys.path.append(_p)

import numpy as np
import ml_dtypes

import concourse.bass as bass
import concourse.tile as tile
from concourse.tile import add_dep_helper
from concourse import bacc, mybir
from concourse import bass_utils
from concourse.bass_utils import run_bass_kernel_spmd

# If a caller enables tracing (BASS_TRACE=1), artifact upload may have no
# bucket access in this container; fall back to the local dir.
_orig_upload = bass_utils.upload_artifacts


def _safe_upload(tmpdir):
    try:
        return _orig_upload(tmpdir)
    except Exception:
        return tmpdir


bass_utils.upload_artifacts = _safe_upload

B, T, D = 2, 2048, 2048
H, KV, HD = 16, 4, 128
NR = H // KV  # 4 q heads per kv group
NCORES = 8
ROPE_BASE = 10000.0
SCALE = float(HD) ** -0.5

F32R = mybir.dt.float32r
F32 = mybir.dt.float32
BF16 = mybir.dt.bfloat16
NBF = ml_dtypes.bfloat16

_cache = {}


def _build_nc():
    nc = bacc.Bacc("TRN2", target_bir_lowering=False, debug=False,
                   num_devices=NCORES)

    xt_e = nc.dram_tensor("xt", [128, 16, T], BF16, kind="ExternalInput").ap()
    wqt_e = [nc.dram_tensor(f"wqt{j}", [128, 16, HD], BF16,
                            kind="ExternalInput").ap() for j in range(NR)]
    wkt_e = nc.dram_tensor("wkt", [128, 16, HD], BF16, kind="ExternalInput").ap()
    wvt_e = nc.dram_tensor("wvt", [128, 16, HD], BF16, kind="ExternalInput").ap()
    wot_e = nc.dram_tensor("wot", [128, NR, D], BF16, kind="ExternalInput").ap()
    cos_e = nc.dram_tensor("cosa", [128, T], F32R, kind="ExternalInput").ap()
    sin_e = nc.dram_tensor("sina", [128, T], F32R, kind="ExternalInput").ap()
    ident_e = nc.dram_tensor("ident", [128, 128], BF16, kind="ExternalInput").ap()
    ones_e = nc.dram_tensor("ones", [128, 128], BF16, kind="ExternalInput").ap()
    out_e = nc.dram_tensor("out", [T, D], BF16, kind="ExternalOutput").ap()

    with tile.TileContext(nc) as tc:
        import contextlib
        with contextlib.ExitStack() as ctx:
            consts = ctx.enter_context(tc.tile_pool(name="consts", bufs=1))
            weights = ctx.enter_context(tc.tile_pool(name="weights", bufs=1))
            acts = ctx.enter_context(tc.tile_pool(name="acts", bufs=1))

            cos_sb = consts.tile([128, T], F32R, tag="cos")
            sin_sb = consts.tile([128, T], F32R, tag="sin")
            ident_sb = consts.tile([128, 128], BF16, tag="ident")
            ones_sb = consts.tile([128, 128], BF16, tag="ones")
            wkt_sb = weights.tile([128, 16, HD], BF16, tag="wkt")
            wvt_sb = weights.tile([128, 16, HD], BF16, tag="wvt")
            wqt_sb = [weights.tile([128, 16, HD], BF16, tag=f"wqt{j}",
                                   name=f"wqt{j}_sb") for j in range(NR)]
            wot_sb = weights.tile([128, NR, D], BF16, tag="wot")
            # DMA ordering: the first k-proj matmul needs wkt chunk 0 and the
            # first x slice — put them FIRST on two different HWDGE queues so
            # they transfer in parallel. Everything else follows; gpsimd
            # SWDGE takes bulk weights + consts.
            nc.sync.dma_start(out=wkt_sb[:, :2, :], in_=wkt_e[:, :2, :])
            dwv = nc.gpsimd.dma_start(out=wvt_sb, in_=wvt_e)
            # cos/sin first halves early on gpsimd: needed by the first
            # RoPE, ~4us after the first matmul
            nc.gpsimd.dma_start(out=cos_sb[:, :512], in_=cos_e[:, :512])
            nc.gpsimd.dma_start(out=sin_sb[:, :512], in_=sin_e[:, :512])
            gated_dmas = []
            for j in range(NR):
                dq = nc.gpsimd.dma_start(out=wqt_sb[j], in_=wqt_e[j])
                gated_dmas.append((dq.ins, max(0, 3 * j - 1)))

            qtr = [acts.tile([128, T], BF16, tag=f"qtr{j}", name=f"qtr{j}")
                   for j in range(NR)]
            ktr = acts.tile([128, T], BF16, tag="ktr")
            v_sb = acts.tile([128, 16, HD], BF16, tag="vsb")  # v natural, s-chunked

            # ---------------- Phase 1: projections + RoPE + v transpose ----
            with tc.tile_pool(name="xt", bufs=12) as xt_pool, \
                 tc.tile_pool(name="rope", bufs=2) as rope_pool, \
                 tc.tile_pool(name="p1ps", bufs=1, space="PSUM") as p1ps, \
                 tc.tile_pool(name="rotps", bufs=2, space="PSUM") as rotps:
                anchors = []  # tt0 k-proj matmul instructions

                def issue_x(tt):
                    # DMA one t-chunk of x, striped across both HWDGE queues
                    tsl = slice(tt * 512, (tt + 1) * 512)
                    xq = []
                    for i in range(4):
                        xti = xt_pool.tile([128, 4, 512], BF16, tag="xt")
                        lo = i * 4
                        if tt == 0 and i == 0:
                            # split so the first k matmul waits on 128KB
                            # only, on the otherwise-idle scalar queue
                            nc.scalar.dma_start(out=xti[:, 0:1, :],
                                                in_=xt_e[:, 0:1, tsl])
                            nc.sync.dma_start(out=wkt_sb[:, 2:, :],
                                              in_=wkt_e[:, 2:, :])
                            nc.scalar.dma_start(out=xti[:, 1:4, :],
                                                in_=xt_e[:, 1:4, tsl])
                        else:
                            nc.sync.dma_start(out=xti[:, 0:2, :],
                                              in_=xt_e[:, lo:lo + 2, tsl])
                            nc.scalar.dma_start(out=xti[:, 2:4, :],
                                                in_=xt_e[:, lo + 2:lo + 4, tsl])
                        xq.append(xti)
                    return xq

                # 2-tile-deep x prefetch: tt and tt+1 in flight while tt
                # computes; tt+2 issued at the top of tt's body. The small
                # ident/ones consts slot in on the scalar queue after each
                # x chunk.
                xq_pend = [issue_x(0)]
                nc.scalar.dma_start(out=ident_sb, in_=ident_e)
                xq_pend.append(issue_x(1))
                nc.scalar.dma_start(out=ones_sb, in_=ones_e)
                # second halves of cos/sin (needed from tt=1 RoPE on)
                nc.gpsimd.dma_start(out=cos_sb[:, 512:], in_=cos_e[:, 512:])
                nc.gpsimd.dma_start(out=sin_sb[:, 512:], in_=sin_e[:, 512:])
                for tt in range(4):
                    tsl = slice(tt * 512, (tt + 1) * 512)
                    if tt + 2 < 4:
                        xq_pend.append(issue_x(tt + 2))
                    xq = xq_pend.pop(0)
                    qps = [p1ps.tile([128, 512], F32, tag=f"qps{j}",
                                     name=f"qps{j}_{tt}") for j in range(NR)]
                    kps = p1ps.tile([128, 512], F32, tag="kps")
                    vps = p1ps.tile([128, 512], F32, tag="vps")
                    for i in range(4):
                        for dc in range(4):
                            g = i * 4 + dc
                            mk = nc.tensor.matmul(kps, wkt_sb[:, g, :],
                                                  xq[i][:, dc, :],
                                                  start=(g == 0), stop=(g == 15))
                            if tt == 0:
                                anchors.append(mk.ins)
                    for i in range(4):
                        for dc in range(4):
                            g = i * 4 + dc
                            nc.tensor.matmul(vps, wvt_sb[:, g, :], xq[i][:, dc, :],
                                             start=(g == 0), stop=(g == 15))
                    for j in range(NR):
                        for i in range(4):
                            for dc in range(4):
                                g = i * 4 + dc
                                nc.tensor.matmul(
                                    qps[j], wqt_sb[j][:, g, :],
                                    xq[i][:, dc, :], start=(g == 0), stop=(g == 15))

                    # RoPE: dst = src*cos + rotate_half(src)*sin, muls on DVE
                    # via partition-shifted PSUM reads (sign of the lower
                    # half folded into the host sin table), add on Pool.
                    def rope(src, dst):
                        t1 = rope_pool.tile([128, 512], BF16, tag="t1", name="t1")
                        nc.vector.tensor_mul(t1, src, cos_sb[:, tsl])
                        t2 = rope_pool.tile([128, 512], BF16, tag="t2", name="t2")
                        nc.vector.tensor_mul(t2[0:64, :], src[64:128, :],
                                             sin_sb[0:64, tsl])
                        nc.vector.tensor_mul(t2[64:128, :], src[0:64, :],
                                             sin_sb[64:128, tsl])
                        nc.vector.tensor_add(dst, t1, t2)

                    rope(kps, ktr[:, tsl])
                    # v: copy vT psum -> sbuf bf16, PE-transpose 128-blocks
                    vt_sb = rope_pool.tile([128, 512], BF16, tag="vt")
                    nc.scalar.copy(vt_sb, vps)
                    for vb in range(4):
                        tr_ps = rotps.tile([128, 128], BF16, tag="rot")
                        nc.tensor.transpose(tr_ps, vt_sb[:, vb * 128:(vb + 1) * 128],
                                            ident_sb)
                        nc.vector.tensor_copy(v_sb[:, tt * 4 + vb, :], tr_ps)
                    for j in range(NR):
                        rope(qps[j], qtr[j][:, tsl])

                for dins, aidx in gated_dmas:
                    add_dep_helper(dins, anchors[min(aidx, 15)],
                                   reason="gate bulk dma behind startup")

            # ---------------- Phase 2+3: attention + out projection --------
            dwot = nc.gpsimd.dma_start(out=wot_sb, in_=wot_e)
            add_dep_helper(dwot.ins, anchors[15], reason="gate wot dma")
            with tc.tile_pool(name="p2sb", bufs=6) as p2sb, \
                 tc.tile_pool(name="dens", bufs=3) as dens, \
                 tc.tile_pool(name="otn", bufs=2) as otnp, \
                 tc.tile_pool(name="ostg", bufs=4) as ostg, \
                 tc.tile_pool(name="stps", bufs=2, space="PSUM") as stps, \
                 tc.tile_pool(name="otps", bufs=2, space="PSUM") as otps, \
                 tc.tile_pool(name="outps", bufs=2, space="PSUM") as outps:
                pending = [None]    # deferred softmax epilogue of previous head
                pend_out = []       # deferred out-projection pieces (prev tt)

                def flush_epilogue():
                    if pending[0] is not None:
                        pending[0]()
                        pending[0] = None

                def out_piece(tt, tkc, otn_t, dts):
                    # half a t-chunk of the out projection: 8 matmuls + evacs
                    rows = slice(tt * 512 + tkc * 128, tt * 512 + (tkc + 1) * 128)
                    for dt in dts:
                        o_ps = outps.tile([128, 512], F32, tag="ops",
                                          name=f"o_ps_{tt}_{tkc}_{dt}")
                        for hh in range(NR):
                            nc.tensor.matmul(
                                o_ps, otn_t[:, hh, tkc * 128:(tkc + 1) * 128],
                                wot_sb[:, hh, dt * 512:(dt + 1) * 512],
                                start=(hh == 0), stop=(hh == NR - 1))
                        o_sb = ostg.tile([128, 512], BF16, tag="ostg",
                                         name=f"o_sb_{tt}_{tkc}_{dt}")
                        # evac split DVE/ACT: DVE saturates at head
                        # boundaries (den adds + dsum + rden + normalize),
                        # so the dt>=2 pieces (popped at steps 7/8) cast on
                        # ACT, which has ~2.6us/head of slack after its exps
                        if dt >= 2:
                            nc.scalar.copy(o_sb, o_ps)
                        else:
                            nc.vector.tensor_copy(o_sb, o_ps)
                        # all output DMA triggers ride the SP queue: the
                        # scalar queue serializes with ACT's exp stream and
                        # Pool SWDGE measured slower for these too
                        nc.sync.dma_start(
                            out=out_e[rows, dt * 512:(dt + 1) * 512], in_=o_sb)

                for tt in range(4):
                    tsl = slice(tt * 512, (tt + 1) * 512)
                    otn_t = otnp.tile([128, NR, 512], BF16, tag="otn")
                    for h in range(NR):
                        ot_ps = otps.tile([128, 512], F32, tag="ot",
                                          name=f"ot_{tt}_{h}")
                        den = dens.tile([128, 2, 512], BF16, tag="den",
                                        name=f"den_{tt}_{h}")
                        exs = {}
                        # one-deep software pipeline: ST(sg) runs one step
                        # ahead of PV(sg) so PE never waits on the exp
                        for step in range(9):
                            if step < 8:
                                st_ps = stps.tile([128, 2, 512], F32, tag="st",
                                                  name=f"st_{tt}_{h}_{step}")
                                for half in range(2):
                                    sc = step * 2 + half
                                    nc.tensor.matmul(
                                        st_ps[:, half, :],
                                        ktr[:, sc * 128:(sc + 1) * 128],
                                        qtr[h][:, tsl], start=True, stop=True)
                                ex = p2sb.tile([128, 2, 512], BF16, tag="exp",
                                               name=f"ex_{tt}_{h}_{step}")
                                nc.scalar.activation(
                                    ex, st_ps, mybir.ActivationFunctionType.Exp,
                                    scale=SCALE)
                                exs[step] = ex
                                # denominator chunk accumulation on DVE;
                                # all-bf16 operands hit the 2x perf mode
                                if step == 1:
                                    nc.vector.tensor_add(den, exs[0], exs[1])
                                elif step > 1:
                                    nc.vector.tensor_add(den, den, ex)
                            if step == 8:
                                # merge the two half-denominators on DVE as
                                # early as possible (right after the step-7
                                # den add in DVE queue order) so the PE
                                # partition-reduce in the epilogue never
                                # stalls on a backlogged DVE
                                dsum = dens.tile([128, 512], BF16, tag="dsum",
                                                 name=f"dsum_{tt}_{h}")
                                nc.vector.tensor_add(dsum, den[:, 0, :],
                                                     den[:, 1, :])
                            if step >= 1:
                                sg = step - 1
                                for half in range(2):
                                    sc = sg * 2 + half
                                    nc.tensor.matmul(ot_ps, v_sb[:, sc, :],
                                                     exs[sg][:, half, :],
                                                     start=(sc == 0),
                                                     stop=(sc == 15))
                                if sg > 1:
                                    exs.pop(sg - 2, None)
                            if step == 2:
                                # previous head's epilogue: overlaps this
                                # head's score stream
                                flush_epilogue()
                            if step in (3, 5, 7, 8) and pend_out:
                                # a piece of the previous t-tile's out
                                # projection as PE filler
                                pend_out.pop(0)()

                        def epilogue(ot_ps=ot_ps, dsum=dsum, h=h,
                                     otn_t=otn_t, tt=tt):
                            # partition-reduce+broadcast denominator on PE
                            bc_ps = outps.tile([128, 512], F32, tag="ops",
                                               name=f"bc_{tt}_{h}")
                            nc.tensor.matmul(bc_ps, ones_sb, dsum,
                                             start=True, stop=True)
                            rden = dens.tile([128, 512], F32, tag="rden",
                                             name=f"rden_{tt}_{h}")
                            nc.vector.reciprocal_approx_fast(rden, bc_ps)
                            nc.vector.tensor_tensor(out=otn_t[:, h, :], in0=ot_ps,
                                                    in1=rden,
                                                    op=mybir.AluOpType.mult)
                        pending[0] = epilogue

                    flush_epilogue()
                    pend_out = [
                        (lambda tt=tt, tkc=tkc, otn_t=otn_t, dts=dts:
                         out_piece(tt, tkc, otn_t, dts))
                        for tkc in range(4) for dts in ((0,), (1,), (2,), (3,))]
                # final t-tile's out projection
                for p in pend_out:
                    p()
    nc.compile()
    return nc


def _get_nc():
    if "nc" not in _cache:
        _cache["nc"] = _build_nc()
    return _cache["nc"]


def _host_consts():
    if "consts" in _cache:
        return _cache["consts"]
    inv = 1.0 / (ROPE_BASE ** (np.arange(0, HD, 2, dtype=np.float64) / HD))
    freqs = np.outer(np.arange(T, dtype=np.float64), inv)  # [T, 64]
    emb = np.concatenate([freqs, freqs], axis=-1)  # [T, 128]
    cos_t = np.cos(emb).T.astype(np.float32).copy()  # [128, T]
    sin_t = np.sin(emb).T.astype(np.float32).copy()
    sin_t[:64, :] *= -1.0  # rotate-half sign folded in (see rope())
    ident = np.eye(128, dtype=np.float32).astype(NBF)
    ones = np.ones((128, 128), dtype=np.float32).astype(NBF)
    _cache["consts"] = (cos_t, sin_t, ident, ones)
    return _cache["consts"]


def _in_maps(x, wq, wk, wv, wo):
    cos_t, sin_t, ident, ones = _host_consts()
    maps = []
    for c in range(NCORES):
        b, g = c // KV, c % KV
        xt = np.ascontiguousarray(
            x[b].reshape(T, 16, 128).transpose(2, 1, 0)).astype(NBF)
        wq_g = wq[g * NR * HD:(g + 1) * NR * HD]  # [512, D]
        # per-head contiguous slices: wqt{j}[p, dc, jc] = wq_g[j*128+jc, dc*128+p]
        wq_h = wq_g.reshape(NR, HD, 16, 128).transpose(0, 3, 2, 1)  # [j, p, dc, jc]
        wk_g = wk[g * HD:(g + 1) * HD]
        wkt = np.ascontiguousarray(wk_g.reshape(HD, 16, 128).transpose(2, 1, 0))
        wv_g = wv[g * HD:(g + 1) * HD]
        wvt = np.ascontiguousarray(wv_g.reshape(HD, 16, 128).transpose(2, 1, 0))
        wo_g = wo[:, g * NR * HD:(g + 1) * NR * HD]  # [D, 512]
        wot = np.ascontiguousarray(
            wo_g.reshape(D, NR, 128).transpose(2, 1, 0)).astype(NBF)
        m = {
            "xt": xt, "wkt": wkt.astype(NBF),
            "wvt": wvt.astype(NBF), "wot": wot,
            "cosa": cos_t, "sina": sin_t,
            "ident": ident, "ones": ones,
        }
        for j in range(NR):
            m[f"wqt{j}"] = np.ascontiguousarray(wq_h[j]).astype(NBF)
        maps.append(m)
    return maps


def run_spmd(x, wq, wk, wv, wo, **kw):
    nc = _get_nc()
    maps = _in_maps(x, wq, wk, wv, wo)
    return run_bass_kernel_spmd(nc, maps, core_ids=list(range(NCORES)), **kw)


def kernel(x, wq, wk, wv, wo):
    res = run_spmd(x, wq, wk, wv, wo)
    out = np.zeros((B, T, D), dtype=np.float32)
    for c in range(NCORES):
        out[c // KV] += res.results[c]["out"].astype(np.float32)
    return out



# revision 18
# speedup vs baseline: 1.0137x; 1.0137x over previous
"""GQA attention kernel for 8 TRN2 NeuronCores.

Problem: B=2, T=2048, D=2048, H=16 q-heads, KV=4 kv-heads, HD=128, RoPE,
non-causal softmax, out projection. f32 reference, rel-err gate 2e-2.

Sharding: 8 cores = 2 batches x 4 kv-groups. Core c handles batch c//4 and
kv-group c%4 (4 q heads + 1 kv head). Each core computes a partial output
x @ wq_g -> attention -> (heads g) @ wo_g^T: full [T, D] partial summed on
host over the 4 groups of each batch (tensor-parallel unshard).

On-device layout: everything transposed ([hd, t], hd=128=partition dim).
All matmul operands are bf16 (HW forbids mixing 32-bit with 16-bit
operands): halved input DMA, Fast Weight Load on every stationary
operand, and 2x DVE throughput for the bf16 softmax-denominator
accumulation. The output is DMA'd as bf16 and accumulated in f32 on the
host (halves output traffic; ~1e-4 extra error). fp8/DoubleRow was
measured numerically unusable (>2.5% error per quantized operand pair
vs the 2e-2 budget); this all-bf16 layout lands at ~9.7e-3.

Engine placement (measured, not theoretical): PE runs all matmuls
(~270us busy of ~301us span, the binding resource); ACT runs only the
softmax exp (135us; its ~1054ns/tile paces the attention inner loop, so
out-projection pieces are interleaved 4x per head as PE filler); DVE
does RoPE, denominator adds, normalize, and all PSUM evacuation. The
Pool engine is left idle: on HW it takes ~2.2us per 1024-elem tensor op
(sim models it fast) and it cannot access PSUM. All output DMA triggers
ride the SP queue - triggers on the scalar queue serialize with ACT's
exp instruction stream.
"""
import os
import sys

for _p in ("/opt/trn_rl_repo", "/root/.axon_site/_ro/trn_rl_repo"):
    if os.path.isdir(_p) and _p not in sys.path:
        sys.path.append(_p)

import numpy as np
import ml_dtypes

import concourse.bass as bass
import concourse.tile as tile
from concourse.tile import add_dep_helper
from concourse import bacc, mybir
from concourse import bass_utils
from concourse.bass_utils import run_bass_kernel_spmd

# If a caller enables tracing (BASS_TRACE=1), artifact upload may have no
# bucket access in this container; fall back to the local dir.
_orig_upload = bass_utils.upload_artifacts


def _safe_upload(tmpdir):
    try:
        return _orig_upload(tmpdir)
    except Exception:
        return tmpdir


bass_utils.upload_artifacts = _safe_upload

B, T, D = 2, 2048, 2048
H, KV, HD = 16, 4, 128
NR = H // KV  # 4 q heads per kv group
NCORES = 8
ROPE_BASE = 10000.0
SCALE = float(HD) ** -0.5

F32R = mybir.dt.float32r
F32 = mybir.dt.float32
BF16 = mybir.dt.bfloat16
NBF = ml_dtypes.bfloat16

_cache = {}


def _build_nc():
    nc = bacc.Bacc("TRN2", target_bir_lowering=False, debug=False,
                   num_devices=NCORES)

    xt_e = nc.dram_tensor("xt", [128, 16, T], BF16, kind="ExternalInput").ap()
    wqt_e = [nc.dram_tensor(f"wqt{j}", [128, 16, HD], BF16,
                            kind="ExternalInput").ap() for j in range(NR)]
    wkt_e = nc.dram_tensor("wkt", [128, 16, HD], BF16, kind="ExternalInput").ap()
    wvt_e = nc.dram_tensor("wvt", [128, 16, HD], BF16, kind="ExternalInput").ap()
    wot_e = nc.dram_tensor("wot", [128, NR, D], BF16, kind="ExternalInput").ap()
    cos_e = nc.dram_tensor("cosa", [128, T], BF16, kind="ExternalInput").ap()
    sin_e = nc.dram_tensor("sina", [128, T], BF16, kind="ExternalInput").ap()
    ident_e = nc.dram_tensor("ident", [128, 128], BF16, kind="ExternalInput").ap()
    ones_e = nc.dram_tensor("ones", [128, 128], BF16, kind="ExternalInput").ap()
    out_e = nc.dram_tensor("out", [T, D], BF16, kind="ExternalOutput").ap()

    with tile.TileContext(nc) as tc:
        import contextlib
        with contextlib.ExitStack() as ctx:
            consts = ctx.enter_context(tc.tile_pool(name="consts", bufs=1))
            weights = ctx.enter_context(tc.tile_pool(name="weights", bufs=1))
            acts = ctx.enter_context(tc.tile_pool(name="acts", bufs=1))

            # bf16 cos/sin halve their HBM traffic (1MB -> 0.5MB): the
            # startup window is HBM-bandwidth-saturated (x + weights +
            # consts all stream during tt0) and bf16 rounding of cos/sin
            # adds ~0.4% relative error on q/k, well within budget
            cos_sb = consts.tile([128, T], BF16, tag="cos")
            sin_sb = consts.tile([128, T], BF16, tag="sin")
            ident_sb = consts.tile([128, 128], BF16, tag="ident")
            ones_sb = consts.tile([128, 128], BF16, tag="ones")
            wkt_sb = weights.tile([128, 16, HD], BF16, tag="wkt")
            wvt_sb = weights.tile([128, 16, HD], BF16, tag="wvt")
            wqt_sb = [weights.tile([128, 16, HD], BF16, tag=f"wqt{j}",
                                   name=f"wqt{j}_sb") for j in range(NR)]
            wot_sb = weights.tile([128, NR, D], BF16, tag="wot")
            # DMA ordering: the first k-proj matmul needs wkt chunk 0 and the
            # first x slice — put them FIRST on two different HWDGE queues so
            # they transfer in parallel. Everything else follows; gpsimd
            # SWDGE takes bulk weights + consts.
            nc.sync.dma_start(out=wkt_sb[:, :2, :], in_=wkt_e[:, :2, :])
            dwv = nc.gpsimd.dma_start(out=wvt_sb, in_=wvt_e)
            # cos/sin first halves early on gpsimd: needed by the first
            # RoPE, ~4us after the first matmul
            nc.gpsimd.dma_start(out=cos_sb[:, :512], in_=cos_e[:, :512])
            nc.gpsimd.dma_start(out=sin_sb[:, :512], in_=sin_e[:, :512])
            # wqt j is first needed by tt0's q-head-j matmul block (matmul
            # 32+16j of tt0); gate each transfer just far enough ahead so
            # the 2MB of wqt doesn't saturate HBM during tt0's x stream
            gated_dmas = []
            for j in range(NR):
                dq = nc.gpsimd.dma_start(out=wqt_sb[j], in_=wqt_e[j])
                gated_dmas.append((dq.ins, 8 + 16 * j))

            qtr = [acts.tile([128, T], BF16, tag=f"qtr{j}", name=f"qtr{j}")
                   for j in range(NR)]
            ktr = acts.tile([128, T], BF16, tag="ktr")
            v_sb = acts.tile([128, 16, HD], BF16, tag="vsb")  # v natural, s-chunked

            # ---------------- Phase 1: projections + RoPE + v transpose ----
            with tc.tile_pool(name="xt", bufs=12) as xt_pool, \
                 tc.tile_pool(name="rope", bufs=2) as rope_pool, \
                 tc.tile_pool(name="p1ps", bufs=1, space="PSUM") as p1ps, \
                 tc.tile_pool(name="rotps", bufs=2, space="PSUM") as rotps:
                anchors = []  # all 96 tt0 matmul instructions, in order

                def issue_x(tt):
                    # DMA one t-chunk of x, striped across both HWDGE queues
                    tsl = slice(tt * 512, (tt + 1) * 512)
                    xq = []
                    for i in range(4):
                        xti = xt_pool.tile([128, 4, 512], BF16, tag="xt")
                        lo = i * 4
                        if tt == 0 and i == 0:
                            # split so the first k matmul waits on 128KB
                            # only, on the otherwise-idle scalar queue
                            nc.scalar.dma_start(out=xti[:, 0:1, :],
                                                in_=xt_e[:, 0:1, tsl])
                            nc.sync.dma_start(out=wkt_sb[:, 2:, :],
                                              in_=wkt_e[:, 2:, :])
                            nc.scalar.dma_start(out=xti[:, 1:4, :],
                                                in_=xt_e[:, 1:4, tsl])
                        else:
                            nc.sync.dma_start(out=xti[:, 0:2, :],
                                              in_=xt_e[:, lo:lo + 2, tsl])
                            nc.scalar.dma_start(out=xti[:, 2:4, :],
                                                in_=xt_e[:, lo + 2:lo + 4, tsl])
                        xq.append(xti)
                    return xq

                # 2-tile-deep x prefetch: tt and tt+1 in flight while tt
                # computes; tt+2 issued at the top of tt's body. The small
                # ident/ones consts slot in on the scalar queue after each
                # x chunk.
                xq_pend = [issue_x(0)]
                nc.scalar.dma_start(out=ident_sb, in_=ident_e)
                xq_pend.append(issue_x(1))
                nc.scalar.dma_start(out=ones_sb, in_=ones_e)
                # second halves of cos/sin: needed from tt=1's RoPE
                # (~36us); gated late so they stay out of tt0's HBM window
                dcr = nc.gpsimd.dma_start(out=cos_sb[:, 512:], in_=cos_e[:, 512:])
                gated_dmas.append((dcr.ins, 80))
                dsr = nc.gpsimd.dma_start(out=sin_sb[:, 512:], in_=sin_e[:, 512:])
                gated_dmas.append((dsr.ins, 80))
                for tt in range(4):
                    tsl = slice(tt * 512, (tt + 1) * 512)
                    if tt + 2 < 4:
                        xq_pend.append(issue_x(tt + 2))
                    xq = xq_pend.pop(0)
                    qps = [p1ps.tile([128, 512], F32, tag=f"qps{j}",
                                     name=f"qps{j}_{tt}") for j in range(NR)]
                    kps = p1ps.tile([128, 512], F32, tag="kps")
                    vps = p1ps.tile([128, 512], F32, tag="vps")
                    for i in range(4):
                        for dc in range(4):
                            g = i * 4 + dc
                            mk = nc.tensor.matmul(kps, wkt_sb[:, g, :],
                                                  xq[i][:, dc, :],
                                                  start=(g == 0), stop=(g == 15))
                            if tt == 0:
                                anchors.append(mk.ins)
                    for i in range(4):
                        for dc in range(4):
                            g = i * 4 + dc
                            mv = nc.tensor.matmul(vps, wvt_sb[:, g, :],
                                                  xq[i][:, dc, :],
                                                  start=(g == 0), stop=(g == 15))
                            if tt == 0:
                                anchors.append(mv.ins)
                    for j in range(NR):
                        for i in range(4):
                            for dc in range(4):
                                g = i * 4 + dc
                                mq = nc.tensor.matmul(
                                    qps[j], wqt_sb[j][:, g, :],
                                    xq[i][:, dc, :], start=(g == 0), stop=(g == 15))
                                if tt == 0:
                                    anchors.append(mq.ins)
                    if tt == 1:
                        tt1_last_q = mq.ins

                    # RoPE: dst = src*cos + rotate_half(src)*sin, muls on DVE
                    # via partition-shifted PSUM reads (sign of the lower
                    # half folded into the host sin table), add on Pool.
                    def rope(src, dst):
                        t1 = rope_pool.tile([128, 512], BF16, tag="t1", name="t1")
                        nc.vector.tensor_mul(t1, src, cos_sb[:, tsl])
                        t2 = rope_pool.tile([128, 512], BF16, tag="t2", name="t2")
                        nc.vector.tensor_mul(t2[0:64, :], src[64:128, :],
                                             sin_sb[0:64, tsl])
                        nc.vector.tensor_mul(t2[64:128, :], src[0:64, :],
                                             sin_sb[64:128, tsl])
                        nc.vector.tensor_add(dst, t1, t2)

                    rope(kps, ktr[:, tsl])
                    # v: copy vT psum -> sbuf bf16, PE-transpose 128-blocks.
                    # The copy rides DVE, NOT ACT: the ACT queue carries x
                    # DMA triggers in phase 1, and a copy queued behind
                    # ring-throttled triggers head-of-line blocks the PE
                    # transposes (measured 6us stall)
                    vt_sb = rope_pool.tile([128, 512], BF16, tag="vt")
                    nc.vector.tensor_copy(vt_sb, vps)
                    for vb in range(4):
                        tr_ps = rotps.tile([128, 128], BF16, tag="rot")
                        nc.tensor.transpose(tr_ps, vt_sb[:, vb * 128:(vb + 1) * 128],
                                            ident_sb)
                        nc.vector.tensor_copy(v_sb[:, tt * 4 + vb, :], tr_ps)
                    for j in range(NR):
                        rope(qps[j], qtr[j][:, tsl])

                for dins, aidx in gated_dmas:
                    add_dep_helper(dins, anchors[min(aidx, 95)],
                                   reason="gate bulk dma behind startup")

            # ---------------- Phase 2+3: attention + out projection --------
            # wot (2MB) is first needed ~110us in; gate it behind tt1's last
            # q matmul (~52us) so its transfer stays clear of tt0/tt1
            dwot = nc.gpsimd.dma_start(out=wot_sb, in_=wot_e)
            add_dep_helper(dwot.ins, tt1_last_q, reason="gate wot dma")
            with tc.tile_pool(name="p2sb", bufs=6) as p2sb, \
                 tc.tile_pool(name="dens", bufs=3) as dens, \
                 tc.tile_pool(name="otn", bufs=2) as otnp, \
                 tc.tile_pool(name="ostg", bufs=4) as ostg, \
                 tc.tile_pool(name="stps", bufs=2, space="PSUM") as stps, \
                 tc.tile_pool(name="otps", bufs=2, space="PSUM") as otps, \
                 tc.tile_pool(name="outps", bufs=2, space="PSUM") as outps:
                pending = [None]    # deferred softmax epilogue of previous head
                pend_out = []       # deferred out-projection pieces (prev tt)

                def flush_epilogue():
                    if pending[0] is not None:
                        pending[0]()
                        pending[0] = None

                def out_piece(tt, tkc, otn_t, dts):
                    # half a t-chunk of the out projection: 8 matmuls + evacs
                    rows = slice(tt * 512 + tkc * 128, tt * 512 + (tkc + 1) * 128)
                    for dt in dts:
                        o_ps = outps.tile([128, 512], F32, tag="ops",
                                          name=f"o_ps_{tt}_{tkc}_{dt}")
                        for hh in range(NR):
                            nc.tensor.matmul(
                                o_ps, otn_t[:, hh, tkc * 128:(tkc + 1) * 128],
                                wot_sb[:, hh, dt * 512:(dt + 1) * 512],
                                start=(hh == 0), stop=(hh == NR - 1))
                        o_sb = ostg.tile([128, 512], BF16, tag="ostg",
                                         name=f"o_sb_{tt}_{tkc}_{dt}")
                        # evac split DVE/ACT: DVE saturates at head
                        # boundaries (den adds + dsum + rden + normalize),
                        # so the dt>=2 pieces (popped at steps 7/8) cast on
                        # ACT, which has ~2.6us/head of slack after its exps
                        if dt >= 2:
                            nc.scalar.copy(o_sb, o_ps)
                        else:
                            nc.vector.tensor_copy(o_sb, o_ps)
                        # output DMA triggers ride the SP queue (the scalar
                        # queue serializes with ACT's exp stream), EXCEPT
                        # the final t-tile: its pieces run after all exps,
                        # so striping across both queues drains the output
                        # backlog before the teardown barrier
                        eng = nc.scalar if (tt == 3 and (tkc + dt) % 2) \
                            else nc.sync
                        eng.dma_start(
                            out=out_e[rows, dt * 512:(dt + 1) * 512], in_=o_sb)

                for tt in range(4):
                    tsl = slice(tt * 512, (tt + 1) * 512)
                    otn_t = otnp.tile([128, NR, 512], BF16, tag="otn")
                    for h in range(NR):
                        ot_ps = otps.tile([128, 512], F32, tag="ot",
                                          name=f"ot_{tt}_{h}")
                        den = dens.tile([128, 2, 512], BF16, tag="den",
                                        name=f"den_{tt}_{h}")
                        exs = {}
                        # one-deep software pipeline: ST(sg) runs one step
                        # ahead of PV(sg) so PE never waits on the exp
                        for step in range(9):
                            if step < 8:
                                st_ps = stps.tile([128, 2, 512], F32, tag="st",
                                                  name=f"st_{tt}_{h}_{step}")
                                for half in range(2):
                                    sc = step * 2 + half
                                    nc.tensor.matmul(
                                        st_ps[:, half, :],
                                        ktr[:, sc * 128:(sc + 1) * 128],
                                        qtr[h][:, tsl], start=True, stop=True)
                                ex = p2sb.tile([128, 2, 512], BF16, tag="exp",
                                               name=f"ex_{tt}_{h}_{step}")
                                nc.scalar.activation(
                                    ex, st_ps, mybir.ActivationFunctionType.Exp,
                                    scale=SCALE)
                                exs[step] = ex
                                # denominator chunk accumulation on DVE;
                                # all-bf16 operands hit the 2x perf mode
                                if step == 1:
                                    nc.vector.tensor_add(den, exs[0], exs[1])
                                elif step > 1:
                                    nc.vector.tensor_add(den, den, ex)
                            if step == 8:
                                # merge the two half-denominators on DVE as
                                # early as possible (right after the step-7
                                # den add in DVE queue order) so the PE
                                # partition-reduce in the epilogue never
                                # stalls on a backlogged DVE
                                dsum = dens.tile([128, 512], BF16, tag="dsum",
                                                 name=f"dsum_{tt}_{h}")
                                nc.vector.tensor_add(dsum, den[:, 0, :],
                                                     den[:, 1, :])
                            if step >= 1:
                                sg = step - 1
                                for half in range(2):
                                    sc = sg * 2 + half
                                    nc.tensor.matmul(ot_ps, v_sb[:, sc, :],
                                                     exs[sg][:, half, :],
                                                     start=(sc == 0),
                                                     stop=(sc == 15))
                                if sg > 1:
                                    exs.pop(sg - 2, None)
                            if step == 2:
                                # previous head's epilogue: overlaps this
                                # head's score stream
                                flush_epilogue()
                            if step in (3, 5, 7, 8) and pend_out:
                                # a piece of the previous t-tile's out
                                # projection as PE filler
                                pend_out.pop(0)()

                        def epilogue(ot_ps=ot_ps, dsum=dsum, h=h,
                                     otn_t=otn_t, tt=tt):
                            # partition-reduce+broadcast denominator on PE
                            bc_ps = outps.tile([128, 512], F32, tag="ops",
                                               name=f"bc_{tt}_{h}")
                            nc.tensor.matmul(bc_ps, ones_sb, dsum,
                                             start=True, stop=True)
                            rden = dens.tile([128, 512], F32, tag="rden",
                                             name=f"rden_{tt}_{h}")
                            nc.vector.reciprocal_approx_fast(rden, bc_ps)
                            nc.vector.tensor_tensor(out=otn_t[:, h, :], in0=ot_ps,
                                                    in1=rden,
                                                    op=mybir.AluOpType.mult)
                        pending[0] = epilogue

                    flush_epilogue()
                    pend_out = [
                        (lambda tt=tt, tkc=tkc, otn_t=otn_t, dts=dts:
                         out_piece(tt, tkc, otn_t, dts))
                        for tkc in range(4) for dts in ((0,), (1,), (2,), (3,))]
                # final t-tile's out projection
                for p in pend_out:
                    p()
    nc.compile()
    return nc


def _get_nc():
    if "nc" not in _cache:
        _cache["nc"] = _build_nc()
    return _cache["nc"]


def _host_consts():
    if "consts" in _cache:
        return _cache["consts"]
    inv = 1.0 / (ROPE_BASE ** (np.arange(0, HD, 2, dtype=np.float64) / HD))
    freqs = np.outer(np.arange(T, dtype=np.float64), inv)  # [T, 64]
    emb = np.concatenate([freqs, freqs], axis=-1)  # [T, 128]
    cos_t = np.cos(emb).T.astype(np.float32).copy()  # [128, T]
    sin_t = np.sin(emb).T.astype(np.float32).copy()
    sin_t[:64, :] *= -1.0  # rotate-half sign folded in (see rope())
    cos_t = cos_t.astype(NBF)
    sin_t = sin_t.astype(NBF)
    ident = np.eye(128, dtype=np.float32).astype(NBF)
    ones = np.ones((128, 128), dtype=np.float32).astype(NBF)
    _cache["consts"] = (cos_t, sin_t, ident, ones)
    return _cache["consts"]


def _in_maps(x, wq, wk, wv, wo):
    cos_t, sin_t, ident, ones = _host_consts()
    maps = []
    for c in range(NCORES):
        b, g = c // KV, c % KV
        xt = np.ascontiguousarray(
            x[b].reshape(T, 16, 128).transpose(2, 1, 0)).astype(NBF)
        wq_g = wq[g * NR * HD:(g + 1) * NR * HD]  # [512, D]
        # per-head contiguous slices: wqt{j}[p, dc, jc] = wq_g[j*128+jc, dc*128+p]
        wq_h = wq_g.reshape(NR, HD, 16, 128).transpose(0, 3, 2, 1)  # [j, p, dc, jc]
        wk_g = wk[g * HD:(g + 1) * HD]
        wkt = np.ascontiguousarray(wk_g.reshape(HD, 16, 128).transpose(2, 1, 0))
        wv_g = wv[g * HD:(g + 1) * HD]
        wvt = np.ascontiguousarray(wv_g.reshape(HD, 16, 128).transpose(2, 1, 0))
        wo_g = wo[:, g * NR * HD:(g + 1) * NR * HD]  # [D, 512]
        wot = np.ascontiguousarray(
            wo_g.reshape(D, NR, 128).transpose(2, 1, 0)).astype(NBF)
        m = {
            "xt": xt, "wkt": wkt.astype(NBF),
            "wvt": wvt.astype(NBF), "wot": wot,
            "cosa": cos_t, "sina": sin_t,
            "ident": ident, "ones": ones,
        }
        for j in range(NR):
            m[f"wqt{j}"] = np.ascontiguousarray(wq_h[j]).astype(NBF)
        maps.append(m)
    return maps


def run_spmd(x, wq, wk, wv, wo, **kw):
    nc = _get_nc()
    maps = _in_maps(x, wq, wk, wv, wo)
    return run_bass_kernel_spmd(nc, maps, core_ids=list(range(NCORES)), **kw)


def kernel(x, wq, wk, wv, wo):
    res = run_spmd(x, wq, wk, wv, wo)
    out = np.zeros((B, T, D), dtype=np.float32)
    for c in range(NCORES):
        out[c // KV] += res.results[c]["out"].astype(np.float32)
    return out



# revision 27
# speedup vs baseline: 1.0448x; 1.0307x over previous
"""GQA attention kernel for 8 TRN2 NeuronCores.

Problem: B=2, T=2048, D=2048, H=16 q-heads, KV=4 kv-heads, HD=128, RoPE,
non-causal softmax, out projection. f32 reference, rel-err gate 2e-2.

Sharding: 8 cores = 2 batches x 4 kv-groups. Core c handles batch c//4 and
kv-group c%4 (4 q heads + 1 kv head). Each core computes a partial output
x @ wq_g -> attention -> (heads g) @ wo_g^T: full [T, D] partial summed on
host over the 4 groups of each batch (tensor-parallel unshard).

On-device layout: everything transposed ([hd, t], hd=128=partition dim).
All matmul operands are bf16 (HW forbids mixing 32-bit with 16-bit
operands): halved input DMA, Fast Weight Load on every stationary
operand, and 2x DVE throughput for the bf16 softmax-denominator
accumulation. The output is DMA'd as bf16 and accumulated in f32 on the
host (halves output traffic; ~1e-4 extra error). fp8/DoubleRow was
measured numerically unusable (>2.5% error per quantized operand pair
vs the 2e-2 budget); this all-bf16 layout lands at ~9.7e-3.

Engine placement (measured, not theoretical): PE runs all matmuls
(~270us busy of ~301us span, the binding resource); ACT runs only the
softmax exp (135us; its ~1054ns/tile paces the attention inner loop, so
out-projection pieces are interleaved 4x per head as PE filler); DVE
does RoPE, denominator adds, normalize, and all PSUM evacuation. The
Pool engine is left idle: on HW it takes ~2.2us per 1024-elem tensor op
(sim models it fast) and it cannot access PSUM. All output DMA triggers
ride the SP queue - triggers on the scalar queue serialize with ACT's
exp instruction stream.
"""
import os
import sys

for _p in ("/opt/trn_rl_repo", "/root/.axon_site/_ro/trn_rl_repo"):
    if os.path.isdir(_p) and _p not in sys.path:
        sys.path.append(_p)

import numpy as np
import ml_dtypes

import concourse.bass as bass
import concourse.tile as tile
from concourse.tile import add_dep_helper
from concourse import bacc, mybir
from concourse import bass_utils
from concourse.bass_utils import run_bass_kernel_spmd

# If a caller enables tracing (BASS_TRACE=1), artifact upload may have no
# bucket access in this container; fall back to the local dir.
_orig_upload = bass_utils.upload_artifacts


def _safe_upload(tmpdir):
    try:
        return _orig_upload(tmpdir)
    except Exception:
        return tmpdir


bass_utils.upload_artifacts = _safe_upload

B, T, D = 2, 2048, 2048
H, KV, HD = 16, 4, 128
NR = H // KV  # 4 q heads per kv group
NCORES = 8
ROPE_BASE = 10000.0
SCALE = float(HD) ** -0.5

F32R = mybir.dt.float32r
F32 = mybir.dt.float32
BF16 = mybir.dt.bfloat16
NBF = ml_dtypes.bfloat16

_cache = {}


def _build_nc():
    nc = bacc.Bacc("TRN2", target_bir_lowering=False, debug=False,
                   num_devices=NCORES)

    xt_e = nc.dram_tensor("xt", [128, 16, T], BF16, kind="ExternalInput").ap()
    wqt_e = [nc.dram_tensor(f"wqt{j}", [128, 16, HD], BF16,
                            kind="ExternalInput").ap() for j in range(NR)]
    wkt_e = nc.dram_tensor("wkt", [128, 16, HD], BF16, kind="ExternalInput").ap()
    wvt_e = nc.dram_tensor("wvt", [128, 16, HD], BF16, kind="ExternalInput").ap()
    wot_e = nc.dram_tensor("wot", [128, NR, D], BF16, kind="ExternalInput").ap()
    cos_e = nc.dram_tensor("cosa", [128, T], BF16, kind="ExternalInput").ap()
    sin_e = nc.dram_tensor("sina", [128, T], BF16, kind="ExternalInput").ap()
    ident_e = nc.dram_tensor("ident", [128, 128], BF16, kind="ExternalInput").ap()
    ones_e = nc.dram_tensor("ones", [128, 128], BF16, kind="ExternalInput").ap()
    out_e = nc.dram_tensor("out", [T, D], BF16, kind="ExternalOutput").ap()

    with tile.TileContext(nc) as tc:
        import contextlib
        with contextlib.ExitStack() as ctx:
            consts = ctx.enter_context(tc.tile_pool(name="consts", bufs=1))
            weights = ctx.enter_context(tc.tile_pool(name="weights", bufs=1))
            acts = ctx.enter_context(tc.tile_pool(name="acts", bufs=1))

            # bf16 cos/sin halve their HBM traffic (1MB -> 0.5MB): the
            # startup window is HBM-bandwidth-saturated (x + weights +
            # consts all stream during tt0) and bf16 rounding of cos/sin
            # adds ~0.4% relative error on q/k, well within budget
            cos_sb = consts.tile([128, T], BF16, tag="cos")
            sin_sb = consts.tile([128, T], BF16, tag="sin")
            ident_sb = consts.tile([128, 128], BF16, tag="ident")
            ones_sb = consts.tile([128, 128], BF16, tag="ones")
            wkt_sb = weights.tile([128, 16, HD], BF16, tag="wkt")
            wvt_sb = weights.tile([128, 16, HD], BF16, tag="wvt")
            wqt_sb = [weights.tile([128, 16, HD], BF16, tag=f"wqt{j}",
                                   name=f"wqt{j}_sb") for j in range(NR)]
            wot_sb = weights.tile([128, NR, D], BF16, tag="wot")
            # DMA ordering: the first k-proj matmul needs wkt chunk 0 and the
            # first x slice — put them FIRST on two different HWDGE queues so
            # they transfer in parallel. Everything else follows; gpsimd
            # SWDGE takes bulk weights + consts.
            nc.sync.dma_start(out=wkt_sb[:, :2, :], in_=wkt_e[:, :2, :])
            dwv = nc.gpsimd.dma_start(out=wvt_sb, in_=wvt_e)
            # cos/sin first halves early on gpsimd: needed by the first
            # RoPE, ~4us after the first matmul
            nc.gpsimd.dma_start(out=cos_sb[:, :512], in_=cos_e[:, :512])
            nc.gpsimd.dma_start(out=sin_sb[:, :512], in_=sin_e[:, :512])
            # wqt j is first needed by tt0's q-head-j matmul block (matmul
            # 32+16j of tt0); gate each transfer just far enough ahead so
            # the 2MB of wqt doesn't saturate HBM during tt0's x stream
            gated_dmas = []
            for j in range(NR):
                dq = nc.gpsimd.dma_start(out=wqt_sb[j], in_=wqt_e[j])
                gated_dmas.append((dq.ins, 8 + 16 * j))

            qtr = [acts.tile([128, T], BF16, tag=f"qtr{j}", name=f"qtr{j}")
                   for j in range(NR)]
            ktr = acts.tile([128, T], BF16, tag="ktr")
            v_sb = acts.tile([128, 16, HD], BF16, tag="vsb")  # v natural, s-chunked

            # ---------------- Phase 1: projections + RoPE + v transpose ----
            # xt_pool lives in the outer scope: tt3's x tiles feed the
            # q(tt3) projections that run as PE filler inside phase 2
            xt_pool = ctx.enter_context(tc.tile_pool(name="xt", bufs=12))
            with tc.tile_pool(name="rope", bufs=2) as rope_pool, \
                 tc.tile_pool(name="p1ps", bufs=1, space="PSUM") as p1ps, \
                 tc.tile_pool(name="rotps", bufs=2, space="PSUM") as rotps:
                anchors = []  # all 96 tt0 matmul instructions, in order

                def issue_x(tt):
                    # DMA one t-chunk of x, striped across both HWDGE queues
                    tsl = slice(tt * 512, (tt + 1) * 512)
                    xq = []
                    for i in range(4):
                        xti = xt_pool.tile([128, 4, 512], BF16, tag="xt")
                        lo = i * 4
                        if tt == 0 and i == 0:
                            # per-chunk transfers so each k matmul g waits
                            # on only its own 128KB, on the otherwise-idle
                            # scalar queue; wkt rest split likewise
                            for dc in range(4):
                                nc.scalar.dma_start(out=xti[:, dc:dc + 1, :],
                                                    in_=xt_e[:, dc:dc + 1, tsl])
                            nc.sync.dma_start(out=wkt_sb[:, 2:8, :],
                                              in_=wkt_e[:, 2:8, :])
                            nc.sync.dma_start(out=wkt_sb[:, 8:, :],
                                              in_=wkt_e[:, 8:, :])
                        else:
                            nc.sync.dma_start(out=xti[:, 0:2, :],
                                              in_=xt_e[:, lo:lo + 2, tsl])
                            nc.scalar.dma_start(out=xti[:, 2:4, :],
                                                in_=xt_e[:, lo + 2:lo + 4, tsl])
                        xq.append(xti)
                    return xq

                # 2-tile-deep x prefetch: tt and tt+1 in flight while tt
                # computes; tt+2 issued at the top of tt's body. The small
                # ident/ones consts slot in on the scalar queue after each
                # x chunk.
                xq_pend = [issue_x(0)]
                nc.scalar.dma_start(out=ident_sb, in_=ident_e)
                xq_pend.append(issue_x(1))
                nc.scalar.dma_start(out=ones_sb, in_=ones_e)
                # second halves of cos/sin: needed from tt=1's RoPE
                # (~36us); gated late so they stay out of tt0's HBM window
                dcr = nc.gpsimd.dma_start(out=cos_sb[:, 512:], in_=cos_e[:, 512:])
                gated_dmas.append((dcr.ins, 80))
                dsr = nc.gpsimd.dma_start(out=sin_sb[:, 512:], in_=sin_e[:, 512:])
                gated_dmas.append((dsr.ins, 80))
                for tt in range(4):
                    tsl = slice(tt * 512, (tt + 1) * 512)
                    if tt + 2 < 4:
                        xq_pend.append(issue_x(tt + 2))
                    xq = xq_pend.pop(0)
                    qps = [p1ps.tile([128, 512], F32, tag=f"qps{j}",
                                     name=f"qps{j}_{tt}") for j in range(NR)] \
                        if tt < 3 else None
                    kps = p1ps.tile([128, 512], F32, tag="kps")
                    vps = p1ps.tile([128, 512], F32, tag="vps")
                    for i in range(4):
                        for dc in range(4):
                            g = i * 4 + dc
                            mk = nc.tensor.matmul(kps, wkt_sb[:, g, :],
                                                  xq[i][:, dc, :],
                                                  start=(g == 0), stop=(g == 15))
                            if tt == 0:
                                anchors.append(mk.ins)
                    for i in range(4):
                        for dc in range(4):
                            g = i * 4 + dc
                            mv = nc.tensor.matmul(vps, wvt_sb[:, g, :],
                                                  xq[i][:, dc, :],
                                                  start=(g == 0), stop=(g == 15))
                            if tt == 0:
                                anchors.append(mv.ins)
                    if tt < 3:
                        for j in range(NR):
                            for i in range(4):
                                for dc in range(4):
                                    g = i * 4 + dc
                                    mq = nc.tensor.matmul(
                                        qps[j], wqt_sb[j][:, g, :],
                                        xq[i][:, dc, :],
                                        start=(g == 0), stop=(g == 15))
                                    if tt == 0:
                                        anchors.append(mq.ins)
                        if tt == 1:
                            tt1_last_q = mq.ins
                    else:
                        # tt3's q projections are deferred into phase 2:
                        # they run as PE filler inside tt=0's attention
                        # heads (which otherwise idle behind ACT's exp)
                        xq3 = xq

                    # RoPE: dst = src*cos + rotate_half(src)*sin, muls on DVE
                    # via partition-shifted PSUM reads (sign of the lower
                    # half folded into the host sin table), add on Pool.
                    def rope(src, dst):
                        t1 = rope_pool.tile([128, 512], BF16, tag="t1", name="t1")
                        nc.vector.tensor_mul(t1, src, cos_sb[:, tsl])
                        t2 = rope_pool.tile([128, 512], BF16, tag="t2", name="t2")
                        nc.vector.tensor_mul(t2[0:64, :], src[64:128, :],
                                             sin_sb[0:64, tsl])
                        nc.vector.tensor_mul(t2[64:128, :], src[0:64, :],
                                             sin_sb[64:128, tsl])
                        nc.vector.tensor_add(dst, t1, t2)

                    rope(kps, ktr[:, tsl])
                    # v: copy vT psum -> sbuf bf16, PE-transpose 128-blocks.
                    # The copy rides DVE, NOT ACT: the ACT queue carries x
                    # DMA triggers in phase 1, and a copy queued behind
                    # ring-throttled triggers head-of-line blocks the PE
                    # transposes (measured 6us stall)
                    vt_sb = rope_pool.tile([128, 512], BF16, tag="vt")
                    nc.vector.tensor_copy(vt_sb, vps)
                    for vb in range(4):
                        tr_ps = rotps.tile([128, 128], BF16, tag="rot")
                        nc.tensor.transpose(tr_ps, vt_sb[:, vb * 128:(vb + 1) * 128],
                                            ident_sb)
                        nc.vector.tensor_copy(v_sb[:, tt * 4 + vb, :], tr_ps)
                    if tt < 3:
                        for j in range(NR):
                            rope(qps[j], qtr[j][:, tsl])

                for dins, aidx in gated_dmas:
                    add_dep_helper(dins, anchors[min(aidx, 95)],
                                   reason="gate bulk dma behind startup")

            # ---------------- Phase 2+3: attention + out projection --------
            # wot (2MB) is first needed ~110us in; gate it behind tt1's last
            # q matmul (~52us) so its transfer stays clear of tt0/tt1
            dwot = nc.gpsimd.dma_start(out=wot_sb, in_=wot_e)
            add_dep_helper(dwot.ins, tt1_last_q, reason="gate wot dma")
            with tc.tile_pool(name="p2sb", bufs=6) as p2sb, \
                 tc.tile_pool(name="dens", bufs=3) as dens, \
                 tc.tile_pool(name="otn", bufs=2) as otnp, \
                 tc.tile_pool(name="ostg", bufs=4) as ostg, \
                 tc.tile_pool(name="q3r", bufs=2) as q3r, \
                 tc.tile_pool(name="stps", bufs=2, space="PSUM") as stps, \
                 tc.tile_pool(name="otps", bufs=2, space="PSUM") as otps, \
                 tc.tile_pool(name="outps", bufs=2, space="PSUM") as outps:
                pending = [None]    # deferred softmax epilogue of previous head
                pend_out = []       # deferred out-projection pieces (prev tt)

                def flush_epilogue():
                    if pending[0] is not None:
                        pending[0]()
                        pending[0] = None

                def out_piece(tt, tkc, otn_t, dts):
                    # half a t-chunk of the out projection: 8 matmuls + evacs
                    rows = slice(tt * 512 + tkc * 128, tt * 512 + (tkc + 1) * 128)
                    for dt in dts:
                        o_ps = outps.tile([128, 512], F32, tag="ops",
                                          name=f"o_ps_{tt}_{tkc}_{dt}")
                        for hh in range(NR):
                            nc.tensor.matmul(
                                o_ps, otn_t[:, hh, tkc * 128:(tkc + 1) * 128],
                                wot_sb[:, hh, dt * 512:(dt + 1) * 512],
                                start=(hh == 0), stop=(hh == NR - 1))
                        o_sb = ostg.tile([128, 512], BF16, tag="ostg",
                                         name=f"o_sb_{tt}_{tkc}_{dt}")
                        # evac split DVE/ACT: DVE saturates at head
                        # boundaries (den adds + dsum + rden + normalize),
                        # so the dt>=2 pieces (popped at steps 7/8) cast on
                        # ACT, which has ~2.6us/head of slack after its exps
                        if dt >= 2:
                            nc.scalar.copy(o_sb, o_ps)
                        else:
                            nc.vector.tensor_copy(o_sb, o_ps)
                        # output DMA triggers ride the SP queue (the scalar
                        # queue serializes with ACT's exp stream), EXCEPT
                        # the final t-tile: its pieces run after all exps,
                        # so striping across both queues drains the output
                        # backlog before the teardown barrier
                        eng = nc.scalar if (tt == 3 and (tkc + dt) % 2) \
                            else nc.sync
                        eng.dma_start(
                            out=out_e[rows, dt * 512:(dt + 1) * 512], in_=o_sb)

                TSL3 = slice(3 * 512, 4 * 512)
                for tt in range(4):
                    tsl = slice(tt * 512, (tt + 1) * 512)
                    otn_t = otnp.tile([128, NR, 512], BF16, tag="otn")
                    for h in range(NR):
                        ot_ps = otps.tile([128, 512], F32, tag="ot",
                                          name=f"ot_{tt}_{h}")
                        den = dens.tile([128, 2, 512], BF16, tag="den",
                                        name=f"den_{tt}_{h}")
                        if tt == 0:
                            # deferred q(tt3) projection for head h: its 16
                            # matmuls run as PE filler inside this head's
                            # exp-paced step loop (tt=0 has no out-proj
                            # pieces to fill with)
                            q3_ps = outps.tile([128, 512], F32, tag="ops",
                                               name=f"q3ps_{h}")
                        exs = {}
                        # one-deep software pipeline: ST(sg) runs one step
                        # ahead of PV(sg) so PE never waits on the exp
                        for step in range(9):
                            if step < 8:
                                st_ps = stps.tile([128, 2, 512], F32, tag="st",
                                                  name=f"st_{tt}_{h}_{step}")
                                for half in range(2):
                                    sc = step * 2 + half
                                    nc.tensor.matmul(
                                        st_ps[:, half, :],
                                        ktr[:, sc * 128:(sc + 1) * 128],
                                        qtr[h][:, tsl], start=True, stop=True)
                                ex = p2sb.tile([128, 2, 512], BF16, tag="exp",
                                               name=f"ex_{tt}_{h}_{step}")
                                nc.scalar.activation(
                                    ex, st_ps, mybir.ActivationFunctionType.Exp,
                                    scale=SCALE)
                                exs[step] = ex
                                # denominator chunk accumulation on DVE;
                                # all-bf16 operands hit the 2x perf mode
                                if step == 1:
                                    nc.vector.tensor_add(den, exs[0], exs[1])
                                elif step > 1:
                                    nc.vector.tensor_add(den, den, ex)
                            if step == 8:
                                # merge the two half-denominators on DVE as
                                # early as possible (right after the step-7
                                # den add in DVE queue order) so the PE
                                # partition-reduce in the epilogue never
                                # stalls on a backlogged DVE
                                dsum = dens.tile([128, 512], BF16, tag="dsum",
                                                 name=f"dsum_{tt}_{h}")
                                nc.vector.tensor_add(dsum, den[:, 0, :],
                                                     den[:, 1, :])
                            if step >= 1 and tt == 0:
                                for half in range(2):
                                    g = (step - 1) * 2 + half
                                    nc.tensor.matmul(
                                        q3_ps, wqt_sb[h][:, g, :],
                                        xq3[g // 4][:, g % 4, :],
                                        start=(g == 0), stop=(g == 15))
                            if step >= 1:
                                sg = step - 1
                                for half in range(2):
                                    sc = sg * 2 + half
                                    nc.tensor.matmul(ot_ps, v_sb[:, sc, :],
                                                     exs[sg][:, half, :],
                                                     start=(sc == 0),
                                                     stop=(sc == 15))
                                if sg > 1:
                                    exs.pop(sg - 2, None)
                            if step == 2:
                                # previous head's epilogue: overlaps this
                                # head's score stream
                                flush_epilogue()
                            if step in (3, 5, 7, 8) and pend_out:
                                # a piece of the previous t-tile's out
                                # projection as PE filler
                                pend_out.pop(0)()

                        if tt == 0:
                            # RoPE the deferred q(tt3) head into qtr (DVE
                            # has ~2.6us/head of slack in tt=0's loop)
                            t1 = q3r.tile([128, 512], BF16, tag="t1",
                                          name=f"q3t1_{h}")
                            nc.vector.tensor_mul(t1, q3_ps, cos_sb[:, TSL3])
                            t2 = q3r.tile([128, 512], BF16, tag="t2",
                                          name=f"q3t2_{h}")
                            nc.vector.tensor_mul(t2[0:64, :], q3_ps[64:128, :],
                                                 sin_sb[0:64, TSL3])
                            nc.vector.tensor_mul(t2[64:128, :], q3_ps[0:64, :],
                                                 sin_sb[64:128, TSL3])
                            nc.vector.tensor_add(qtr[h][:, TSL3], t1, t2)

                        def epilogue(ot_ps=ot_ps, dsum=dsum, h=h,
                                     otn_t=otn_t, tt=tt):
                            # partition-reduce+broadcast denominator on PE
                            bc_ps = outps.tile([128, 512], F32, tag="ops",
                                               name=f"bc_{tt}_{h}")
                            nc.tensor.matmul(bc_ps, ones_sb, dsum,
                                             start=True, stop=True)
                            rden = dens.tile([128, 512], F32, tag="rden",
                                             name=f"rden_{tt}_{h}")
                            nc.vector.reciprocal_approx_fast(rden, bc_ps)
                            nc.vector.tensor_tensor(out=otn_t[:, h, :], in0=ot_ps,
                                                    in1=rden,
                                                    op=mybir.AluOpType.mult)
                        pending[0] = epilogue

                    flush_epilogue()
                    pend_out = [
                        (lambda tt=tt, tkc=tkc, otn_t=otn_t, dts=dts:
                         out_piece(tt, tkc, otn_t, dts))
                        for tkc in range(4) for dts in ((0,), (1,), (2,), (3,))]
                # final t-tile's out projection
                for p in pend_out:
                    p()
    nc.compile()
    return nc


def _get_nc():
    if "nc" not in _cache:
        _cache["nc"] = _build_nc()
    return _cache["nc"]


def _host_consts():
    if "consts" in _cache:
        return _cache["consts"]
    inv = 1.0 / (ROPE_BASE ** (np.arange(0, HD, 2, dtype=np.float64) / HD))
    freqs = np.outer(np.arange(T, dtype=np.float64), inv)  # [T, 64]
    emb = np.concatenate([freqs, freqs], axis=-1)  # [T, 128]
    cos_t = np.cos(emb).T.astype(np.float32).copy()  # [128, T]
    sin_t = np.sin(emb).T.astype(np.float32).copy()
    sin_t[:64, :] *= -1.0  # rotate-half sign folded in (see rope())
    cos_t = cos_t.astype(NBF)
    sin_t = sin_t.astype(NBF)
    ident = np.eye(128, dtype=np.float32).astype(NBF)
    ones = np.ones((128, 128), dtype=np.float32).astype(NBF)
    _cache["consts"] = (cos_t, sin_t, ident, ones)
    return _cache["consts"]


def _in_maps(x, wq, wk, wv, wo):
    cos_t, sin_t, ident, ones = _host_consts()
    maps = []
    for c in range(NCORES):
        b, g = c // KV, c % KV
        xt = np.ascontiguousarray(
            x[b].reshape(T, 16, 128).transpose(2, 1, 0)).astype(NBF)
        wq_g = wq[g * NR * HD:(g + 1) * NR * HD]  # [512, D]
        # per-head contiguous slices: wqt{j}[p, dc, jc] = wq_g[j*128+jc, dc*128+p]
        wq_h = wq_g.reshape(NR, HD, 16, 128).transpose(0, 3, 2, 1)  # [j, p, dc, jc]
        wk_g = wk[g * HD:(g + 1) * HD]
        wkt = np.ascontiguousarray(wk_g.reshape(HD, 16, 128).transpose(2, 1, 0))
        wv_g = wv[g * HD:(g + 1) * HD]
        wvt = np.ascontiguousarray(wv_g.reshape(HD, 16, 128).transpose(2, 1, 0))
        wo_g = wo[:, g * NR * HD:(g + 1) * NR * HD]  # [D, 512]
        wot = np.ascontiguousarray(
            wo_g.reshape(D, NR, 128).transpose(2, 1, 0)).astype(NBF)
        m = {
            "xt": xt, "wkt": wkt.astype(NBF),
            "wvt": wvt.astype(NBF), "wot": wot,
            "cosa": cos_t, "sina": sin_t,
            "ident": ident, "ones": ones,
        }
        for j in range(NR):
            m[f"wqt{j}"] = np.ascontiguousarray(wq_h[j]).astype(NBF)
        maps.append(m)
    return maps


def run_spmd(x, wq, wk, wv, wo, **kw):
    nc = _get_nc()
    maps = _in_maps(x, wq, wk, wv, wo)
    return run_bass_kernel_spmd(nc, maps, core_ids=list(range(NCORES)), **kw)


def kernel(x, wq, wk, wv, wo):
    res = run_spmd(x, wq, wk, wv, wo)
    out = np.zeros((B, T, D), dtype=np.float32)
    for c in range(NCORES):
        out[c // KV] += res.results[c]["out"].astype(np.float32)
    return out



# revision 30
# speedup vs baseline: 1.0478x; 1.0029x over previous
"""GQA attention kernel for 8 TRN2 NeuronCores.

Problem: B=2, T=2048, D=2048, H=16 q-heads, KV=4 kv-heads, HD=128, RoPE,
non-causal softmax, out projection. f32 reference, rel-err gate 2e-2.

Sharding: 8 cores = 2 batches x 4 kv-groups. Core c handles batch c//4 and
kv-group c%4 (4 q heads + 1 kv head). Each core computes a partial output
x @ wq_g -> attention -> (heads g) @ wo_g^T: full [T, D] partial summed on
host over the 4 groups of each batch (tensor-parallel unshard).

On-device layout: everything transposed ([hd, t], hd=128=partition dim).
All matmul operands are bf16 (HW forbids mixing 32-bit with 16-bit
operands): halved input DMA, Fast Weight Load on every stationary
operand, and 2x DVE throughput for the bf16 softmax-denominator
accumulation. The output is DMA'd as bf16 and accumulated in f32 on the
host (halves output traffic; ~1e-4 extra error). fp8/DoubleRow was
measured numerically unusable (>2.5% error per quantized operand pair
vs the 2e-2 budget); this all-bf16 layout lands at ~9.7e-3.

Engine placement (measured, not theoretical): PE runs all matmuls
(~270us busy of ~301us span, the binding resource); ACT runs only the
softmax exp (135us; its ~1054ns/tile paces the attention inner loop, so
out-projection pieces are interleaved 4x per head as PE filler); DVE
does RoPE, denominator adds, normalize, and all PSUM evacuation. The
Pool engine is left idle: on HW it takes ~2.2us per 1024-elem tensor op
(sim models it fast) and it cannot access PSUM. All output DMA triggers
ride the SP queue - triggers on the scalar queue serialize with ACT's
exp instruction stream.
"""
import os
import sys

for _p in ("/opt/trn_rl_repo", "/root/.axon_site/_ro/trn_rl_repo"):
    if os.path.isdir(_p) and _p not in sys.path:
        sys.path.append(_p)

import numpy as np
import ml_dtypes

import concourse.bass as bass
import concourse.tile as tile
from concourse.tile import add_dep_helper
from concourse import bacc, mybir
from concourse import bass_utils
from concourse.bass_utils import run_bass_kernel_spmd

# If a caller enables tracing (BASS_TRACE=1), artifact upload may have no
# bucket access in this container; fall back to the local dir.
_orig_upload = bass_utils.upload_artifacts


def _safe_upload(tmpdir):
    try:
        return _orig_upload(tmpdir)
    except Exception:
        return tmpdir


bass_utils.upload_artifacts = _safe_upload

B, T, D = 2, 2048, 2048
H, KV, HD = 16, 4, 128
NR = H // KV  # 4 q heads per kv group
NCORES = 8
ROPE_BASE = 10000.0
SCALE = float(HD) ** -0.5

F32R = mybir.dt.float32r
F32 = mybir.dt.float32
BF16 = mybir.dt.bfloat16
NBF = ml_dtypes.bfloat16

_cache = {}


def _build_nc():
    nc = bacc.Bacc("TRN2", target_bir_lowering=False, debug=False,
                   num_devices=NCORES)

    xt_e = nc.dram_tensor("xt", [128, 16, T], BF16, kind="ExternalInput").ap()
    wqt_e = [nc.dram_tensor(f"wqt{j}", [128, 16, HD], BF16,
                            kind="ExternalInput").ap() for j in range(NR)]
    wkt_e = nc.dram_tensor("wkt", [128, 16, HD], BF16, kind="ExternalInput").ap()
    wvt_e = nc.dram_tensor("wvt", [128, 16, HD], BF16, kind="ExternalInput").ap()
    wot_e = nc.dram_tensor("wot", [128, NR, D], BF16, kind="ExternalInput").ap()
    cos_e = nc.dram_tensor("cosa", [128, T], BF16, kind="ExternalInput").ap()
    sin_e = nc.dram_tensor("sina", [128, T], BF16, kind="ExternalInput").ap()
    ident_e = nc.dram_tensor("ident", [128, 128], BF16, kind="ExternalInput").ap()
    ones_e = nc.dram_tensor("ones", [128, 128], BF16, kind="ExternalInput").ap()
    out_e = nc.dram_tensor("out", [T, D], BF16, kind="ExternalOutput").ap()

    with tile.TileContext(nc) as tc:
        import contextlib
        with contextlib.ExitStack() as ctx:
            consts = ctx.enter_context(tc.tile_pool(name="consts", bufs=1))
            weights = ctx.enter_context(tc.tile_pool(name="weights", bufs=1))
            acts = ctx.enter_context(tc.tile_pool(name="acts", bufs=1))

            # bf16 cos/sin halve their HBM traffic (1MB -> 0.5MB): the
            # startup window is HBM-bandwidth-saturated (x + weights +
            # consts all stream during tt0) and bf16 rounding of cos/sin
            # adds ~0.4% relative error on q/k, well within budget
            cos_sb = consts.tile([128, T], BF16, tag="cos")
            sin_sb = consts.tile([128, T], BF16, tag="sin")
            ident_sb = consts.tile([128, 128], BF16, tag="ident")
            ones_sb = consts.tile([128, 128], BF16, tag="ones")
            wkt_sb = weights.tile([128, 16, HD], BF16, tag="wkt")
            wvt_sb = weights.tile([128, 16, HD], BF16, tag="wvt")
            wqt_sb = [weights.tile([128, 16, HD], BF16, tag=f"wqt{j}",
                                   name=f"wqt{j}_sb") for j in range(NR)]
            wot_sb = weights.tile([128, NR, D], BF16, tag="wot")
            # DMA ordering: the first k-proj matmul needs wkt chunk 0 and the
            # first x slice — put them FIRST on two different HWDGE queues so
            # they transfer in parallel. Everything else follows; gpsimd
            # SWDGE takes bulk weights + consts.
            nc.sync.dma_start(out=wkt_sb[:, :2, :], in_=wkt_e[:, :2, :])
            # tt0 runs k/v/q0 chunk-major to flatten the startup HBM demand
            # spike, so v(g)/q0(g) need weight block g//4 early: stream
            # wvt/wqt0 as interleaved 4-chunk blocks on gpsimd
            for b in range(4):
                bs = slice(4 * b, 4 * b + 4)
                nc.gpsimd.dma_start(out=wvt_sb[:, bs, :], in_=wvt_e[:, bs, :])
                nc.gpsimd.dma_start(out=wqt_sb[0][:, bs, :],
                                    in_=wqt_e[0][:, bs, :])
            # cos/sin first halves: needed by the first RoPE (~20us)
            nc.gpsimd.dma_start(out=cos_sb[:, :512], in_=cos_e[:, :512])
            nc.gpsimd.dma_start(out=sin_sb[:, :512], in_=sin_e[:, :512])
            # wqt 1-3 feed tt0's trailing q blocks; gate each just far
            # enough ahead that the transfers stay out of the chunk-major
            # window's HBM budget
            gated_dmas = []
            for j in range(1, NR):
                dq = nc.gpsimd.dma_start(out=wqt_sb[j], in_=wqt_e[j])
                gated_dmas.append((dq.ins, (24, 40, 60)[j - 1]))

            qtr = [acts.tile([128, T], BF16, tag=f"qtr{j}", name=f"qtr{j}")
                   for j in range(NR)]
            ktr = acts.tile([128, T], BF16, tag="ktr")
            v_sb = acts.tile([128, 16, HD], BF16, tag="vsb")  # v natural, s-chunked

            # ---------------- Phase 1: projections + RoPE + v transpose ----
            # xt_pool lives in the outer scope: tt3's x tiles feed the
            # q(tt3) projections that run as PE filler inside phase 2
            xt_pool = ctx.enter_context(tc.tile_pool(name="xt", bufs=12))
            with tc.tile_pool(name="rope", bufs=2) as rope_pool, \
                 tc.tile_pool(name="p1ps", bufs=1, space="PSUM") as p1ps, \
                 tc.tile_pool(name="rotps", bufs=2, space="PSUM") as rotps:
                anchors = []  # all 96 tt0 matmul instructions, in order

                def issue_x(tt):
                    # DMA one t-chunk of x, striped across both HWDGE queues
                    tsl = slice(tt * 512, (tt + 1) * 512)
                    xq = []
                    for i in range(4):
                        xti = xt_pool.tile([128, 4, 512], BF16, tag="xt")
                        lo = i * 4
                        if tt == 0 and i == 0:
                            # per-chunk transfers so each k matmul g waits
                            # on only its own 128KB, on the otherwise-idle
                            # scalar queue; wkt rest split likewise
                            for dc in range(4):
                                nc.scalar.dma_start(out=xti[:, dc:dc + 1, :],
                                                    in_=xt_e[:, dc:dc + 1, tsl])
                            nc.sync.dma_start(out=wkt_sb[:, 2:8, :],
                                              in_=wkt_e[:, 2:8, :])
                            nc.sync.dma_start(out=wkt_sb[:, 8:, :],
                                              in_=wkt_e[:, 8:, :])
                        else:
                            nc.sync.dma_start(out=xti[:, 0:2, :],
                                              in_=xt_e[:, lo:lo + 2, tsl])
                            nc.scalar.dma_start(out=xti[:, 2:4, :],
                                                in_=xt_e[:, lo + 2:lo + 4, tsl])
                        xq.append(xti)
                    return xq

                # 2-tile-deep x prefetch: tt and tt+1 in flight while tt
                # computes; tt+2 issued at the top of tt's body. The small
                # ident/ones consts slot in on the scalar queue after each
                # x chunk.
                xq_pend = [issue_x(0)]
                nc.scalar.dma_start(out=ident_sb, in_=ident_e)
                xq_pend.append(issue_x(1))
                nc.scalar.dma_start(out=ones_sb, in_=ones_e)
                # second halves of cos/sin: needed from tt=1's RoPE
                # (~36us); gated late so they stay out of tt0's HBM window
                dcr = nc.gpsimd.dma_start(out=cos_sb[:, 512:], in_=cos_e[:, 512:])
                gated_dmas.append((dcr.ins, 80))
                dsr = nc.gpsimd.dma_start(out=sin_sb[:, 512:], in_=sin_e[:, 512:])
                gated_dmas.append((dsr.ins, 80))
                for tt in range(4):
                    tsl = slice(tt * 512, (tt + 1) * 512)
                    if tt + 2 < 4:
                        xq_pend.append(issue_x(tt + 2))
                    xq = xq_pend.pop(0)
                    qps = [p1ps.tile([128, 512], F32, tag=f"qps{j}",
                                     name=f"qps{j}_{tt}") for j in range(NR)] \
                        if tt < 3 else None
                    kps = p1ps.tile([128, 512], F32, tag="kps")
                    vps = p1ps.tile([128, 512], F32, tag="vps")
                    if tt == 0:
                        # chunk-major k/v/q0: each arriving 128KB x chunk
                        # unlocks 3 matmuls, flattening the HBM demand
                        # spike that a k-then-v-then-q order creates
                        for i in range(4):
                            for dc in range(4):
                                g = i * 4 + dc
                                for dst, w in ((kps, wkt_sb), (vps, wvt_sb),
                                               (qps[0], wqt_sb[0])):
                                    m = nc.tensor.matmul(
                                        dst, w[:, g, :], xq[i][:, dc, :],
                                        start=(g == 0), stop=(g == 15))
                                    anchors.append(m.ins)
                        for j in range(1, NR):
                            for i in range(4):
                                for dc in range(4):
                                    g = i * 4 + dc
                                    mq = nc.tensor.matmul(
                                        qps[j], wqt_sb[j][:, g, :],
                                        xq[i][:, dc, :],
                                        start=(g == 0), stop=(g == 15))
                                    anchors.append(mq.ins)
                    else:
                        for i in range(4):
                            for dc in range(4):
                                g = i * 4 + dc
                                nc.tensor.matmul(kps, wkt_sb[:, g, :],
                                                 xq[i][:, dc, :],
                                                 start=(g == 0), stop=(g == 15))
                        for i in range(4):
                            for dc in range(4):
                                g = i * 4 + dc
                                nc.tensor.matmul(vps, wvt_sb[:, g, :],
                                                 xq[i][:, dc, :],
                                                 start=(g == 0), stop=(g == 15))
                        if tt < 3:
                            for j in range(NR):
                                for i in range(4):
                                    for dc in range(4):
                                        g = i * 4 + dc
                                        mq = nc.tensor.matmul(
                                            qps[j], wqt_sb[j][:, g, :],
                                            xq[i][:, dc, :],
                                            start=(g == 0), stop=(g == 15))
                            if tt == 1:
                                tt1_last_q = mq.ins
                        else:
                            # tt3's q projections are deferred into phase
                            # 2: they run as PE filler inside tt=0's
                            # attention heads (which otherwise idle behind
                            # ACT's exp)
                            xq3 = xq

                    # RoPE: dst = src*cos + rotate_half(src)*sin, muls on DVE
                    # via partition-shifted PSUM reads (sign of the lower
                    # half folded into the host sin table), add on Pool.
                    def rope(src, dst):
                        t1 = rope_pool.tile([128, 512], BF16, tag="t1", name="t1")
                        nc.vector.tensor_mul(t1, src, cos_sb[:, tsl])
                        t2 = rope_pool.tile([128, 512], BF16, tag="t2", name="t2")
                        nc.vector.tensor_mul(t2[0:64, :], src[64:128, :],
                                             sin_sb[0:64, tsl])
                        nc.vector.tensor_mul(t2[64:128, :], src[0:64, :],
                                             sin_sb[64:128, tsl])
                        nc.vector.tensor_add(dst, t1, t2)

                    rope(kps, ktr[:, tsl])
                    # v: copy vT psum -> sbuf bf16, PE-transpose 128-blocks.
                    # The copy rides DVE, NOT ACT: the ACT queue carries x
                    # DMA triggers in phase 1, and a copy queued behind
                    # ring-throttled triggers head-of-line blocks the PE
                    # transposes (measured 6us stall)
                    vt_sb = rope_pool.tile([128, 512], BF16, tag="vt")
                    nc.vector.tensor_copy(vt_sb, vps)
                    for vb in range(4):
                        tr_ps = rotps.tile([128, 128], BF16, tag="rot")
                        nc.tensor.transpose(tr_ps, vt_sb[:, vb * 128:(vb + 1) * 128],
                                            ident_sb)
                        nc.vector.tensor_copy(v_sb[:, tt * 4 + vb, :], tr_ps)
                    if tt < 3:
                        for j in range(NR):
                            rope(qps[j], qtr[j][:, tsl])

                for dins, aidx in gated_dmas:
                    add_dep_helper(dins, anchors[min(aidx, 95)],
                                   reason="gate bulk dma behind startup")

            # ---------------- Phase 2+3: attention + out projection --------
            # wot (2MB) is first needed ~110us in; gate it behind tt1's last
            # q matmul (~52us) so its transfer stays clear of tt0/tt1
            dwot = nc.gpsimd.dma_start(out=wot_sb, in_=wot_e)
            add_dep_helper(dwot.ins, tt1_last_q, reason="gate wot dma")
            with tc.tile_pool(name="p2sb", bufs=6) as p2sb, \
                 tc.tile_pool(name="dens", bufs=3) as dens, \
                 tc.tile_pool(name="otn", bufs=2) as otnp, \
                 tc.tile_pool(name="ostg", bufs=4) as ostg, \
                 tc.tile_pool(name="q3r", bufs=2) as q3r, \
                 tc.tile_pool(name="stps", bufs=2, space="PSUM") as stps, \
                 tc.tile_pool(name="otps", bufs=2, space="PSUM") as otps, \
                 tc.tile_pool(name="outps", bufs=2, space="PSUM") as outps:
                pending = [None]    # deferred softmax epilogue of previous head
                pend_out = []       # deferred out-projection pieces (prev tt)

                def flush_epilogue():
                    if pending[0] is not None:
                        pending[0]()
                        pending[0] = None

                def out_piece(tt, tkc, otn_t, dts):
                    # half a t-chunk of the out projection: 8 matmuls + evacs
                    rows = slice(tt * 512 + tkc * 128, tt * 512 + (tkc + 1) * 128)
                    for dt in dts:
                        o_ps = outps.tile([128, 512], F32, tag="ops",
                                          name=f"o_ps_{tt}_{tkc}_{dt}")
                        for hh in range(NR):
                            nc.tensor.matmul(
                                o_ps, otn_t[:, hh, tkc * 128:(tkc + 1) * 128],
                                wot_sb[:, hh, dt * 512:(dt + 1) * 512],
                                start=(hh == 0), stop=(hh == NR - 1))
                        o_sb = ostg.tile([128, 512], BF16, tag="ostg",
                                         name=f"o_sb_{tt}_{tkc}_{dt}")
                        # evac split DVE/ACT: DVE saturates at head
                        # boundaries (den adds + dsum + rden + normalize),
                        # but two ACT copies/head make ACT the pacer, so
                        # only the dt==3 piece (popped at step 8) casts on
                        # ACT
                        if dt == 3:
                            nc.scalar.copy(o_sb, o_ps)
                        else:
                            nc.vector.tensor_copy(o_sb, o_ps)
                        # output DMA triggers ride the SP queue (the scalar
                        # queue serializes with ACT's exp stream), EXCEPT
                        # the final t-tile: its pieces run after all exps,
                        # so striping across both queues drains the output
                        # backlog before the teardown barrier
                        eng = nc.scalar if (tt == 3 and (tkc + dt) % 2) \
                            else nc.sync
                        eng.dma_start(
                            out=out_e[rows, dt * 512:(dt + 1) * 512], in_=o_sb)

                TSL3 = slice(3 * 512, 4 * 512)
                for tt in range(4):
                    tsl = slice(tt * 512, (tt + 1) * 512)
                    otn_t = otnp.tile([128, NR, 512], BF16, tag="otn")
                    for h in range(NR):
                        ot_ps = otps.tile([128, 512], F32, tag="ot",
                                          name=f"ot_{tt}_{h}")
                        den = dens.tile([128, 2, 512], BF16, tag="den",
                                        name=f"den_{tt}_{h}")
                        if tt == 0:
                            # deferred q(tt3) projection for head h: its 16
                            # matmuls run as PE filler inside this head's
                            # exp-paced step loop (tt=0 has no out-proj
                            # pieces to fill with)
                            q3_ps = outps.tile([128, 512], F32, tag="ops",
                                               name=f"q3ps_{h}")
                        exs = {}
                        # one-deep software pipeline: ST(sg) runs one step
                        # ahead of PV(sg) so PE never waits on the exp
                        for step in range(9):
                            if step < 8:
                                st_ps = stps.tile([128, 2, 512], F32, tag="st",
                                                  name=f"st_{tt}_{h}_{step}")
                                for half in range(2):
                                    sc = step * 2 + half
                                    nc.tensor.matmul(
                                        st_ps[:, half, :],
                                        ktr[:, sc * 128:(sc + 1) * 128],
                                        qtr[h][:, tsl], start=True, stop=True)
                                ex = p2sb.tile([128, 2, 512], BF16, tag="exp",
                                               name=f"ex_{tt}_{h}_{step}")
                                nc.scalar.activation(
                                    ex, st_ps, mybir.ActivationFunctionType.Exp,
                                    scale=SCALE)
                                exs[step] = ex
                                # denominator chunk accumulation on DVE;
                                # all-bf16 operands hit the 2x perf mode
                                if step == 1:
                                    nc.vector.tensor_add(den, exs[0], exs[1])
                                elif step > 1:
                                    nc.vector.tensor_add(den, den, ex)
                            if step == 8:
                                # merge the two half-denominators on DVE as
                                # early as possible (right after the step-7
                                # den add in DVE queue order) so the PE
                                # partition-reduce in the epilogue never
                                # stalls on a backlogged DVE
                                dsum = dens.tile([128, 512], BF16, tag="dsum",
                                                 name=f"dsum_{tt}_{h}")
                                nc.vector.tensor_add(dsum, den[:, 0, :],
                                                     den[:, 1, :])
                            if step >= 1 and tt == 0:
                                for half in range(2):
                                    g = (step - 1) * 2 + half
                                    nc.tensor.matmul(
                                        q3_ps, wqt_sb[h][:, g, :],
                                        xq3[g // 4][:, g % 4, :],
                                        start=(g == 0), stop=(g == 15))
                            if step >= 1:
                                sg = step - 1
                                for half in range(2):
                                    sc = sg * 2 + half
                                    nc.tensor.matmul(ot_ps, v_sb[:, sc, :],
                                                     exs[sg][:, half, :],
                                                     start=(sc == 0),
                                                     stop=(sc == 15))
                                if sg > 1:
                                    exs.pop(sg - 2, None)
                            if step == 2:
                                # previous head's epilogue: overlaps this
                                # head's score stream
                                flush_epilogue()
                            if step in (3, 5, 7, 8) and pend_out:
                                # a piece of the previous t-tile's out
                                # projection as PE filler
                                pend_out.pop(0)()

                        if tt == 0:
                            # RoPE the deferred q(tt3) head into qtr (DVE
                            # has ~2.6us/head of slack in tt=0's loop)
                            t1 = q3r.tile([128, 512], BF16, tag="t1",
                                          name=f"q3t1_{h}")
                            nc.vector.tensor_mul(t1, q3_ps, cos_sb[:, TSL3])
                            t2 = q3r.tile([128, 512], BF16, tag="t2",
                                          name=f"q3t2_{h}")
                            nc.vector.tensor_mul(t2[0:64, :], q3_ps[64:128, :],
                                                 sin_sb[0:64, TSL3])
                            nc.vector.tensor_mul(t2[64:128, :], q3_ps[0:64, :],
                                                 sin_sb[64:128, TSL3])
                            nc.vector.tensor_add(qtr[h][:, TSL3], t1, t2)

                        def epilogue(ot_ps=ot_ps, dsum=dsum, h=h,
                                     otn_t=otn_t, tt=tt):
                            # partition-reduce+broadcast denominator on PE
                            bc_ps = outps.tile([128, 512], F32, tag="ops",
                                               name=f"bc_{tt}_{h}")
                            nc.tensor.matmul(bc_ps, ones_sb, dsum,
                                             start=True, stop=True)
                            rden = dens.tile([128, 512], F32, tag="rden",
                                             name=f"rden_{tt}_{h}")
                            nc.vector.reciprocal_approx_fast(rden, bc_ps)
                            nc.vector.tensor_tensor(out=otn_t[:, h, :], in0=ot_ps,
                                                    in1=rden,
                                                    op=mybir.AluOpType.mult)
                        pending[0] = epilogue

                    flush_epilogue()
                    pend_out = [
                        (lambda tt=tt, tkc=tkc, otn_t=otn_t, dts=dts:
                         out_piece(tt, tkc, otn_t, dts))
                        for tkc in range(4) for dts in ((0,), (1,), (2,), (3,))]
                # final t-tile's out projection
                for p in pend_out:
                    p()
    nc.compile()
    return nc


def _get_nc():
    if "nc" not in _cache:
        _cache["nc"] = _build_nc()
    return _cache["nc"]


def _host_consts():
    if "consts" in _cache:
        return _cache["consts"]
    inv = 1.0 / (ROPE_BASE ** (np.arange(0, HD, 2, dtype=np.float64) / HD))
    freqs = np.outer(np.arange(T, dtype=np.float64), inv)  # [T, 64]
    emb = np.concatenate([freqs, freqs], axis=-1)  # [T, 128]
    cos_t = np.cos(emb).T.astype(np.float32).copy()  # [128, T]
    sin_t = np.sin(emb).T.astype(np.float32).copy()
    sin_t[:64, :] *= -1.0  # rotate-half sign folded in (see rope())
    cos_t = cos_t.astype(NBF)
    sin_t = sin_t.astype(NBF)
    ident = np.eye(128, dtype=np.float32).astype(NBF)
    ones = np.ones((128, 128), dtype=np.float32).astype(NBF)
    _cache["consts"] = (cos_t, sin_t, ident, ones)
    return _cache["consts"]


def _in_maps(x, wq, wk, wv, wo):
    cos_t, sin_t, ident, ones = _host_consts()
    maps = []
    for c in range(NCORES):
        b, g = c // KV, c % KV
        xt = np.ascontiguousarray(
            x[b].reshape(T, 16, 128).transpose(2, 1, 0)).astype(NBF)
        wq_g = wq[g * NR * HD:(g + 1) * NR * HD]  # [512, D]
        # per-head contiguous slices: wqt{j}[p, dc, jc] = wq_g[j*128+jc, dc*128+p]
        wq_h = wq_g.reshape(NR, HD, 16, 128).transpose(0, 3, 2, 1)  # [j, p, dc, jc]
        wk_g = wk[g * HD:(g + 1) * HD]
        wkt = np.ascontiguousarray(wk_g.reshape(HD, 16, 128).transpose(2, 1, 0))
        wv_g = wv[g * HD:(g + 1) * HD]
        wvt = np.ascontiguousarray(wv_g.reshape(HD, 16, 128).transpose(2, 1, 0))
        wo_g = wo[:, g * NR * HD:(g + 1) * NR * HD]  # [D, 512]
        wot = np.ascontiguousarray(
            wo_g.reshape(D, NR, 128).transpose(2, 1, 0)).astype(NBF)
        m = {
            "xt": xt, "wkt": wkt.astype(NBF),
            "wvt": wvt.astype(NBF), "wot": wot,
            "cosa": cos_t, "sina": sin_t,
            "ident": ident, "ones": ones,
        }
        for j in range(NR):
            m[f"wqt{j}"] = np.ascontiguousarray(wq_h[j]).astype(NBF)
        maps.append(m)
    return maps


def run_spmd(x, wq, wk, wv, wo, **kw):
    nc = _get_nc()
    maps = _in_maps(x, wq, wk, wv, wo)
    return run_bass_kernel_spmd(nc, maps, core_ids=list(range(NCORES)), **kw)


def kernel(x, wq, wk, wv, wo):
    res = run_spmd(x, wq, wk, wv, wo)
    out = np.zeros((B, T, D), dtype=np.float32)
    for c in range(NCORES):
        out[c // KV] += res.results[c]["out"].astype(np.float32)
    return out

